# revision 2
# baseline (speedup 1.0000x reference)
"""BiLSTM Trainium2 kernel, v4 — transposed-z, single-bank merged tail.

Like v3 (z computed transposed, U-stationary, bf16 U/h), but the whole
step's z^T lives in ONE PSUM tile [128, 256] (1 bank):
  columns = [gate block g|i|f|o] x [chunk c 0..3] x [batch 16]
  (gate-major: block gg at cols 64*gg, chunk c at 16*c within the block)
so the step's tail is ONE sigmoid [128,256], ONE tg/fc/ig/cn/hn each at
[128,64], ONE tanh, and h^T comes out as ONE [128,64] bf16 tile whose
16-col slices are exactly the next step's four matmul moving operands.
The xz inject is ONE [128,256] f32r matmul (eye stationary) that opens
the accumulation bank.
"""

import os
import sys

sys.path.insert(0, "/opt/trn_rl_repo")

import numpy as np
import ml_dtypes
from contextlib import ExitStack

import concourse.bass as bass  # noqa: F401
import concourse.tile as tile
from concourse import bacc, mybir
from concourse.bass_utils import run_bass_kernel_spmd

B, T, D, U = 64, 512, 512, 512
G = 4 * U
NCORE = 8
NDIR_CORES = 4
B_LOC = B // NDIR_CORES        # 16
NCHUNK = 4                     # h k-tiles
CH = U // NCHUNK               # 128
NS = G // 128                  # 16 gate slices

ZBUFS = int(os.environ.get("BK4_ZBUFS", "2"))
PJBUFS = int(os.environ.get("BK4_PJBUFS", "2"))
XBUFS = int(os.environ.get("BK4_XBUFS", "4"))
FC_ENG = os.environ.get("BK4_FC_ENG", "dve")
OC_ENG = os.environ.get("BK4_OC_ENG", "dve")
PRE_RT = int(os.environ.get("BK4_PRE_RT", "2"))

F32 = mybir.dt.float32
F32R = mybir.dt.float32r
BF16 = mybir.dt.bfloat16
AF = mybir.ActivationFunctionType

# gate-major slice order: slice s = 4*gg + c, gg in (g, i, f, o)
GBASE = {0: 2 * U, 1: 0, 2: U, 3: 3 * U}  # g, i, f, o original col bases


def _perm_t():
    """permT[128*s + m] = original gate column of slice s, lane m."""
    idx = np.empty(G, np.int64)
    for gg in range(4):
        for c in range(NCHUNK):
            s = 4 * gg + c
            idx[128 * s:128 * (s + 1)] = GBASE[gg] + CH * c + np.arange(128)
    return idx


def _emit(tc, nc, xT, Wp, Up, biasb, eye128, zerosb, hsT, t_steps, b_loc):
    rt = t_steps * b_loc
    n_rt = rt // 512
    W4 = 4 * b_loc              # 64

    with ExitStack() as es:
        consts = es.enter_context(tc.tile_pool(name="consts", bufs=1))
        dramp = es.enter_context(tc.tile_pool(name="dram", bufs=1,
                                              space="DRAM"))

        xzT = dramp.tile([G, rt], F32R, tag="xzT")

        w_t = consts.tile([128, 4, G], F32R, tag="w")
        u_t = consts.tile([128, 4, G], BF16, tag="u")
        for k in range(4):
            nc.sync.dma_start(out=w_t[:, k, :],
                              in_=Wp[128 * k:128 * (k + 1), :])
            nc.sync.dma_start(out=u_t[:, k, :],
                              in_=Up[128 * k:128 * (k + 1), :])
        bias_t = consts.tile([128, NS], F32, tag="bias")
        nc.sync.dma_start(out=bias_t, in_=biasb)
        eye_t = consts.tile([128, 128], F32R, tag="eye")
        nc.sync.dma_start(out=eye_t, in_=eye128)

        # ---- phase 1: xz^T = (x @ W)^T, W stationary -------------------
        es2 = es.enter_context(ExitStack())
        xkp = es2.enter_context(tc.tile_pool(name="xk", bufs=3))
        pcp = es2.enter_context(tc.tile_pool(name="pc", bufs=3))
        pjps = es2.enter_context(tc.tile_pool(name="pjps", bufs=PJBUFS,
                                              space="PSUM"))
        xk_tiles = {}

        def emit_proj_s(rti, s):
            if s == 0:
                xk = xkp.tile([128, 4, 512], F32R, tag="xk",
                              name=f"xk_{rti}")
                for k in range(4):
                    nc.sync.dma_start(
                        out=xk[:, k, :],
                        in_=xT[128 * k:128 * (k + 1),
                               512 * rti:512 * (rti + 1)])
                xk_tiles[rti] = xk
            xk = xk_tiles[rti]
            pj = pjps.tile([128, 512], F32, tag="pj", name=f"pj_{rti}_{s}")
            for k in range(4):
                nc.tensor.matmul(pj, w_t[:, k, 128 * s:128 * (s + 1)],
                                 xk[:, k, :], start=(k == 0), stop=(k == 3))
            oc = pcp.tile([128, 512], F32R, tag="oc", name=f"oc_{rti}_{s}")
            eng = nc.gpsimd if OC_ENG == "pool" else nc.vector
            eng.tensor_scalar(oc, pj, bias_t[:, s:s + 1], 0.0,
                              mybir.AluOpType.add, mybir.AluOpType.bypass)
            nc.sync.dma_start(
                out=xzT[128 * s:128 * (s + 1), 512 * rti:512 * (rti + 1)],
                in_=oc)

        PRE = min(n_rt, PRE_RT)
        for rti in range(PRE):
            for s in range(NS):
                emit_proj_s(rti, s)
        next_q = PRE * NS

        # ---- phase 2: recurrence ---------------------------------------
        with tc.tile_pool(name="xzp", bufs=XBUFS) as xzp, \
             tc.tile_pool(name="state", bufs=2) as state, \
             tc.tile_pool(name="gates", bufs=3) as gp, \
             tc.tile_pool(name="zps", bufs=ZBUFS, space="PSUM") as zps:

            hT_prev = state.tile([128, W4], BF16, tag="hT", name="hT_init")
            c_prev = state.tile([128, W4], F32, tag="c", name="c_init")
            nc.sync.dma_start(out=hT_prev, in_=zerosb[:, :W4])
            nc.vector.memset(c_prev[:], 0.0)

            xz_tiles = {}

            def issue_xz_dma(t2):
                # one DMA loads xz for steps 2*t2 and 2*t2+1
                if 2 * t2 >= t_steps:
                    return
                nb = min(2 * b_loc, (t_steps - 2 * t2) * b_loc)
                xz_t = xzp.tile([128, NS, 2 * b_loc], F32R, tag="xz_t",
                                name=f"xzt_{t2}")
                nc.sync.dma_start(
                    out=xz_t[:, :, 0:nb],
                    in_=xzT[:, 2 * b_loc * t2:2 * b_loc * t2 + nb].rearrange(
                        "(s p) c -> p s c", p=128))
                xz_tiles[t2] = xz_t

            issue_xz_dma(0)
            issue_xz_dma(1)

            for t in range(t_steps):
                xz_t = xz_tiles[t // 2]
                if t % 2 == 1:
                    del xz_tiles[t // 2]
                half = slice(b_loc * (t % 2), b_loc * (t % 2 + 1))
                zt = zps.tile([128, NS * b_loc], F32, tag="z",
                              name=f"z_{t}")
                # the inject opens the whole bank (start=True, no hT dep)
                # ONE start (zeroes the whole psum zero-region) and ONE stop
                # (the very last accumulate) per bank.
                nc.tensor.matmul(
                    zt, eye_t,
                    xz_t[:, :, half],
                    start=True, stop=False,
                    skip_group_check=True)
                for r in range(NCHUNK):
                    last = r == NCHUNK - 1
                    hslice = hT_prev[:, b_loc * r:b_loc * (r + 1)]
                    for s in range(NS):
                        nc.tensor.matmul(
                            zt[:, b_loc * s:b_loc * (s + 1)],
                            u_t[:, r, 128 * s:128 * (s + 1)],
                            hslice,
                            start=False, stop=(last and s == NS - 1),
                            skip_group_check=True)

                # tail: one op per gate stage at [128, 64]
                bg = slice(0, W4)
                bi = slice(W4, 2 * W4)
                bf_ = slice(2 * W4, 3 * W4)
                bo = slice(3 * W4, 4 * W4)
                sig = gp.tile([128, NS * b_loc], F32, tag="sig",
                              name=f"sig_{t}")
                nc.scalar.activation(sig, zt, AF.Sigmoid)
                tg = gp.tile([128, W4], F32, tag="tg", name=f"tg_{t}")
                nc.vector.tensor_scalar(
                    tg, sig[:, bg], 2.0, 1.0,
                    mybir.AluOpType.mult, mybir.AluOpType.subtract)
                fc = gp.tile([128, W4], F32, tag="fc", name=f"fc_{t}")
                fc_eng = nc.gpsimd if FC_ENG == "pool" else nc.vector
                fc_eng.tensor_mul(fc, sig[:, bf_], c_prev)
                ig = gp.tile([128, W4], F32, tag="ig", name=f"ig_{t}")
                nc.vector.tensor_mul(ig, sig[:, bi], tg)
                cn = state.tile([128, W4], F32, tag="c", name=f"c_{t}")
                nc.vector.tensor_add(cn, ig, fc)
                tch = gp.tile([128, W4], F32, tag="tc", name=f"tc_{t}")
                nc.scalar.activation(tch, cn, AF.Tanh)
                hn = state.tile([128, W4], BF16, tag="hT", name=f"hT_{t}")
                nc.vector.tensor_mul(hn, sig[:, bo], tch)
                nc.sync.dma_start(
                    out=hsT[t].rearrange("(c p) b -> p c b", p=128),
                    in_=hn)

                if t % 2 == 0 and next_q < n_rt * NS:
                    emit_proj_s(next_q // NS, next_q % NS)
                    next_q += 1
                if t % 2 == 0:
                    issue_xz_dma(t // 2 + 2)
                hT_prev, c_prev = hn, cn


def build_program(t_steps=T, b_loc=B_LOC):
    rt = t_steps * b_loc
    nc = bacc.Bacc("TRN2", target_bir_lowering=False, debug=False,
                   num_devices=NCORE)
    xT = nc.dram_tensor("xT", [D, rt], F32R, kind="ExternalInput").ap()
    Wp = nc.dram_tensor("Wp", [D, G], F32R, kind="ExternalInput").ap()
    Up = nc.dram_tensor("Up", [U, G], BF16, kind="ExternalInput").ap()
    biasb = nc.dram_tensor("biasb", [128, NS], F32,
                           kind="ExternalInput").ap()
    eye128 = nc.dram_tensor("eye128", [128, 128], F32R,
                            kind="ExternalInput").ap()
    zerosb = nc.dram_tensor("zerosb", [128, 4 * b_loc], BF16,
                            kind="ExternalInput").ap()
    hsT = nc.dram_tensor("hsT", [t_steps, U, b_loc], BF16,
                         kind="ExternalOutput").ap()
    with tile.TileContext(nc) as tc:
        _emit(tc, nc, xT, Wp, Up, biasb, eye128, zerosb, hsT,
              t_steps, b_loc)
    nc.compile()
    return nc


_CACHE = {}


def _get_program(t_steps=T, b_loc=B_LOC):
    key = (t_steps, b_loc)
    if key not in _CACHE:
        _CACHE[key] = build_program(t_steps, b_loc)
    return _CACHE[key]


def make_in_maps(xf, xb, Wf, Uf, bf, Wb, Ub, bb, t_steps=T, b_loc=B_LOC):
    perm = _perm_t()
    gscale = np.ones(G, np.float32)
    gscale[0:4 * 128] = 2.0     # g block is slices 0..3
    packs = {}
    for d, (W, Urec, bias) in enumerate(((Wf, Uf, bf), (Wb, Ub, bb))):
        Wpp = np.ascontiguousarray(W[:, perm] * gscale)
        Upp = np.ascontiguousarray(
            (Urec[:, perm] * gscale).astype(ml_dtypes.bfloat16))
        bp = np.ascontiguousarray(
            ((bias[perm] * gscale).reshape(NS, 128).T).astype(np.float32))
        packs[d] = (Wpp, Upp, bp)
    in_maps = []
    for core in range(NCORE):
        d, j = divmod(core, NDIR_CORES)
        x = (xf if d == 0 else xb)[b_loc * j:b_loc * (j + 1), :t_steps]
        xT = np.ascontiguousarray(
            x.transpose(2, 1, 0).reshape(D, t_steps * b_loc))
        Wpp, Upp, bp = packs[d]
        in_maps.append({
            "xT": xT, "Wp": Wpp, "Up": Upp, "biasb": bp,
            "eye128": np.eye(128, dtype=np.float32),
            "zerosb": np.zeros((128, 4 * b_loc), ml_dtypes.bfloat16)})
    return in_maps


def kernel(xf, xb, Wf, Uf, bf, Wb, Ub, bb):
    xf = np.asarray(xf, np.float32)
    xb = np.asarray(xb, np.float32)
    Wf = np.asarray(Wf, np.float32)
    Uf = np.asarray(Uf, np.float32)
    bf = np.asarray(bf, np.float32)
    Wb = np.asarray(Wb, np.float32)
    Ub = np.asarray(Ub, np.float32)
    bb = np.asarray(bb, np.float32)

    nc = _get_program()
    in_maps = make_in_maps(xf, xb, Wf, Uf, bf, Wb, Ub, bb)
    res = run_bass_kernel_spmd(nc, in_maps, list(range(NCORE)))

    out = np.empty((B, T, 2 * U), np.float32)
    for core in range(NCORE):
        d, j = divmod(core, NDIR_CORES)
        hsv = np.asarray(res.results[core]["hsT"],
                         dtype=np.float32)  # [T, U, b_loc]
        out[B_LOC * j:B_LOC * (j + 1), :, U * d:U * (d + 1)] = \
            hsv.transpose(2, 0, 1)
    return out


# revision 3
# speedup vs baseline: 1.0466x; 1.0466x over previous
"""BiLSTM Trainium2 kernel, v4 — transposed-z, single-bank merged tail.

Like v3 (z computed transposed, U-stationary, bf16 U/h), but the whole
step's z^T lives in ONE PSUM tile [128, 256] (1 bank):
  columns = [gate block g|i|f|o] x [chunk c 0..3] x [batch 16]
  (gate-major: block gg at cols 64*gg, chunk c at 16*c within the block)
so the step's tail is ONE sigmoid [128,256], ONE tg/fc/ig/cn/hn each at
[128,64], ONE tanh, and h^T comes out as ONE [128,64] bf16 tile whose
16-col slices are exactly the next step's four matmul moving operands.
The xz inject is ONE [128,256] f32r matmul (eye stationary) that opens
the accumulation bank.
"""

import os
import sys

sys.path.insert(0, "/opt/trn_rl_repo")

import numpy as np
import ml_dtypes
from contextlib import ExitStack

import concourse.bass as bass  # noqa: F401
import concourse.tile as tile
from concourse import bacc, mybir
from concourse.bass_utils import run_bass_kernel_spmd

B, T, D, U = 64, 512, 512, 512
G = 4 * U
NCORE = 8
NDIR_CORES = 4
B_LOC = B // NDIR_CORES        # 16
NCHUNK = 4                     # h k-tiles
CH = U // NCHUNK               # 128
NS = G // 128                  # 16 gate slices

ZBUFS = int(os.environ.get("BK4_ZBUFS", "2"))
PJBUFS = int(os.environ.get("BK4_PJBUFS", "2"))
XBUFS = int(os.environ.get("BK4_XBUFS", "4"))
FC_ENG = os.environ.get("BK4_FC_ENG", "dve")
OC_ENG = os.environ.get("BK4_OC_ENG", "dve")
PRE_RT = int(os.environ.get("BK4_PRE_RT", "2"))

F32 = mybir.dt.float32
F32R = mybir.dt.float32r
BF16 = mybir.dt.bfloat16
AF = mybir.ActivationFunctionType

# gate-major slice order: slice s = 4*gg + c, gg in (g, i, f, o)
GBASE = {0: 2 * U, 1: 0, 2: U, 3: 3 * U}  # g, i, f, o original col bases


def _perm_t():
    """permT[128*s + m] = original gate column of slice s, lane m.

    Chunk-major: slice s = 4*c + gg, gg in (g, i, f, o).
    """
    idx = np.empty(G, np.int64)
    for c in range(NCHUNK):
        for gg in range(4):
            s = 4 * c + gg
            idx[128 * s:128 * (s + 1)] = GBASE[gg] + CH * c + np.arange(128)
    return idx


def _emit(tc, nc, xT, Wp, Up, biasb, eye128, zerosb, hsT, t_steps, b_loc):
    rt = t_steps * b_loc
    n_rt = rt // 512
    W4 = 4 * b_loc              # 64

    with ExitStack() as es:
        consts = es.enter_context(tc.tile_pool(name="consts", bufs=1))
        dramp = es.enter_context(tc.tile_pool(name="dram", bufs=1,
                                              space="DRAM"))

        xzT = dramp.tile([G, rt], F32R, tag="xzT")

        w_t = consts.tile([128, 4, G], F32R, tag="w")
        u_t = consts.tile([128, 4, G], BF16, tag="u")
        for k in range(4):
            nc.sync.dma_start(out=w_t[:, k, :],
                              in_=Wp[128 * k:128 * (k + 1), :])
            nc.sync.dma_start(out=u_t[:, k, :],
                              in_=Up[128 * k:128 * (k + 1), :])
        bias_t = consts.tile([128, NS], F32, tag="bias")
        nc.sync.dma_start(out=bias_t, in_=biasb)
        eye_t = consts.tile([128, 128], F32R, tag="eye")
        nc.sync.dma_start(out=eye_t, in_=eye128)

        # ---- phase 1: xz^T = (x @ W)^T, W stationary -------------------
        es2 = es.enter_context(ExitStack())
        xkp = es2.enter_context(tc.tile_pool(name="xk", bufs=3))
        pcp = es2.enter_context(tc.tile_pool(name="pc", bufs=3))
        pjps = es2.enter_context(tc.tile_pool(name="pjps", bufs=PJBUFS,
                                              space="PSUM"))
        xk_tiles = {}

        def emit_proj_s(rti, s):
            if s == 0:
                xk = xkp.tile([128, 4, 512], F32R, tag="xk",
                              name=f"xk_{rti}")
                for k in range(4):
                    nc.sync.dma_start(
                        out=xk[:, k, :],
                        in_=xT[128 * k:128 * (k + 1),
                               512 * rti:512 * (rti + 1)])
                xk_tiles[rti] = xk
            xk = xk_tiles[rti]
            pj = pjps.tile([128, 512], F32, tag="pj", name=f"pj_{rti}_{s}")
            for k in range(4):
                nc.tensor.matmul(pj, w_t[:, k, 128 * s:128 * (s + 1)],
                                 xk[:, k, :], start=(k == 0), stop=(k == 3))
            oc = pcp.tile([128, 512], F32R, tag="oc", name=f"oc_{rti}_{s}")
            eng = nc.gpsimd if OC_ENG == "pool" else nc.vector
            eng.tensor_scalar(oc, pj, bias_t[:, s:s + 1], 0.0,
                              mybir.AluOpType.add, mybir.AluOpType.bypass)
            nc.sync.dma_start(
                out=xzT[128 * s:128 * (s + 1), 512 * rti:512 * (rti + 1)],
                in_=oc)

        PRE = min(n_rt, PRE_RT)
        for rti in range(PRE):
            for s in range(NS):
                emit_proj_s(rti, s)
        next_q = PRE * NS

        # ---- phase 2: recurrence ---------------------------------------
        with tc.tile_pool(name="xzp", bufs=XBUFS) as xzp, \
             tc.tile_pool(name="state", bufs=2) as state, \
             tc.tile_pool(name="gates", bufs=3) as gp, \
             tc.tile_pool(name="zps", bufs=ZBUFS, space="PSUM") as zps:

            W2 = 2 * b_loc      # 32: per-half h/c width
            h_prev = [state.tile([128, W2], BF16, tag=f"h{x}",
                                 name=f"h_init{x}") for x in "ab"]
            c_prev = [state.tile([128, W2], F32, tag=f"c{x}",
                                 name=f"c_init{x}") for x in "ab"]
            for x in range(2):
                nc.sync.dma_start(out=h_prev[x], in_=zerosb[:, :W2])
                nc.vector.memset(c_prev[x][:], 0.0)

            xz_tiles = {}

            def issue_xz_dma(t2):
                # one DMA loads xz for steps 2*t2 and 2*t2+1
                if 2 * t2 >= t_steps:
                    return
                nb = min(2 * b_loc, (t_steps - 2 * t2) * b_loc)
                xz_t = xzp.tile([128, NS, 2 * b_loc], F32R, tag="xz_t",
                                name=f"xzt_{t2}")
                nc.sync.dma_start(
                    out=xz_t[:, :, 0:nb],
                    in_=xzT[:, 2 * b_loc * t2:2 * b_loc * t2 + nb].rearrange(
                        "(s p) c -> p s c", p=128))
                xz_tiles[t2] = xz_t

            issue_xz_dma(0)
            issue_xz_dma(1)

            for t in range(t_steps):
                xz_t = xz_tiles[t // 2]
                if t % 2 == 1:
                    del xz_tiles[t // 2]
                half = slice(b_loc * (t % 2), b_loc * (t % 2 + 1))
                # two z banks per step: bank X holds slices of chunk pair X
                # (A: chunks 0-1 / slices 0-7; B: chunks 2-3 / slices 8-15)
                zts = [zps.tile([128, 8 * b_loc], F32, tag=f"z{x}",
                                name=f"z_{t}_{x}") for x in "ab"]
                for X in range(2):
                    nc.tensor.matmul(
                        zts[X], eye_t,
                        xz_t[:, 8 * X:8 * (X + 1), half],
                        start=True, stop=False,
                        skip_group_check=True)
                for r in range(NCHUNK):
                    last = r == NCHUNK - 1
                    hslice = h_prev[r // 2][:, b_loc * (r % 2):
                                            b_loc * (r % 2 + 1)]
                    for s in range(NS):
                        X, sl = divmod(s, 8)
                        nc.tensor.matmul(
                            zts[X][:, b_loc * sl:b_loc * (sl + 1)],
                            u_t[:, r, 128 * s:128 * (s + 1)],
                            hslice,
                            start=False, stop=(last and sl == 7),
                            skip_group_check=True)

                # tails per half: slices within a half are [g|i|f|o] per
                # chunk, so gate planes are stride-4 slice combs.
                h_new = [None, None]
                c_new = [None, None]
                for X in range(2):
                    sig = gp.tile([128, 8 * b_loc], F32, tag=f"sig{X}",
                                  name=f"sig_{t}_{X}")
                    nc.scalar.activation(sig, zts[X], AF.Sigmoid)
                    sg = sig.rearrange("p (s b) -> p s b", b=b_loc)
                    tg = gp.tile([128, W2], F32, tag=f"tg{X}",
                                 name=f"tg_{t}_{X}")
                    nc.vector.tensor_scalar(
                        tg, sg[:, 0::4, :], 2.0, 1.0,
                        mybir.AluOpType.mult, mybir.AluOpType.subtract)
                    fc = gp.tile([128, W2], F32, tag=f"fc{X}",
                                 name=f"fc_{t}_{X}")
                    fc_eng = nc.gpsimd if FC_ENG == "pool" else nc.vector
                    fc_eng.tensor_mul(fc, sg[:, 2::4, :], c_prev[X])
                    ig = gp.tile([128, W2], F32, tag=f"ig{X}",
                                 name=f"ig_{t}_{X}")
                    nc.vector.tensor_mul(ig, sg[:, 1::4, :], tg)
                    cn = state.tile([128, W2], F32, tag=f"c{'ab'[X]}",
                                    name=f"c_{t}_{X}")
                    nc.vector.tensor_add(cn, ig, fc)
                    tch = gp.tile([128, W2], F32, tag=f"tc{X}",
                                  name=f"tc_{t}_{X}")
                    nc.scalar.activation(tch, cn, AF.Tanh)
                    hn = state.tile([128, W2], BF16, tag=f"h{'ab'[X]}",
                                    name=f"h_{t}_{X}")
                    nc.vector.tensor_mul(hn, sg[:, 3::4, :], tch)
                    nc.sync.dma_start(
                        out=hsT[t, 2 * CH * X:2 * CH * (X + 1),
                                :].rearrange("(c p) b -> p c b", p=128),
                        in_=hn)
                    h_new[X] = hn
                    c_new[X] = cn

                if t % 2 == 0 and next_q < n_rt * NS:
                    emit_proj_s(next_q // NS, next_q % NS)
                    next_q += 1
                if t % 2 == 0:
                    issue_xz_dma(t // 2 + 2)
                h_prev, c_prev = h_new, c_new


def build_program(t_steps=T, b_loc=B_LOC):
    rt = t_steps * b_loc
    nc = bacc.Bacc("TRN2", target_bir_lowering=False, debug=False,
                   num_devices=NCORE)
    xT = nc.dram_tensor("xT", [D, rt], F32R, kind="ExternalInput").ap()
    Wp = nc.dram_tensor("Wp", [D, G], F32R, kind="ExternalInput").ap()
    Up = nc.dram_tensor("Up", [U, G], BF16, kind="ExternalInput").ap()
    biasb = nc.dram_tensor("biasb", [128, NS], F32,
                           kind="ExternalInput").ap()
    eye128 = nc.dram_tensor("eye128", [128, 128], F32R,
                            kind="ExternalInput").ap()
    zerosb = nc.dram_tensor("zerosb", [128, 4 * b_loc], BF16,
                            kind="ExternalInput").ap()
    hsT = nc.dram_tensor("hsT", [t_steps, U, b_loc], BF16,
                         kind="ExternalOutput").ap()
    with tile.TileContext(nc) as tc:
        _emit(tc, nc, xT, Wp, Up, biasb, eye128, zerosb, hsT,
              t_steps, b_loc)
    nc.compile()
    return nc


_CACHE = {}


def _get_program(t_steps=T, b_loc=B_LOC):
    key = (t_steps, b_loc)
    if key not in _CACHE:
        _CACHE[key] = build_program(t_steps, b_loc)
    return _CACHE[key]


def make_in_maps(xf, xb, Wf, Uf, bf, Wb, Ub, bb, t_steps=T, b_loc=B_LOC):
    perm = _perm_t()
    gscale = np.ones(G, np.float32)
    for c in range(NCHUNK):
        s = 4 * c               # g slice of chunk c
        gscale[128 * s:128 * (s + 1)] = 2.0
    packs = {}
    for d, (W, Urec, bias) in enumerate(((Wf, Uf, bf), (Wb, Ub, bb))):
        Wpp = np.ascontiguousarray(W[:, perm] * gscale)
        Upp = np.ascontiguousarray(
            (Urec[:, perm] * gscale).astype(ml_dtypes.bfloat16))
        bp = np.ascontiguousarray(
            ((bias[perm] * gscale).reshape(NS, 128).T).astype(np.float32))
        packs[d] = (Wpp, Upp, bp)
    in_maps = []
    for core in range(NCORE):
        d, j = divmod(core, NDIR_CORES)
        x = (xf if d == 0 else xb)[b_loc * j:b_loc * (j + 1), :t_steps]
        xT = np.ascontiguousarray(
            x.transpose(2, 1, 0).reshape(D, t_steps * b_loc))
        Wpp, Upp, bp = packs[d]
        in_maps.append({
            "xT": xT, "Wp": Wpp, "Up": Upp, "biasb": bp,
            "eye128": np.eye(128, dtype=np.float32),
            "zerosb": np.zeros((128, 4 * b_loc), ml_dtypes.bfloat16)})
    return in_maps


def kernel(xf, xb, Wf, Uf, bf, Wb, Ub, bb):
    xf = np.asarray(xf, np.float32)
    xb = np.asarray(xb, np.float32)
    Wf = np.asarray(Wf, np.float32)
    Uf = np.asarray(Uf, np.float32)
    bf = np.asarray(bf, np.float32)
    Wb = np.asarray(Wb, np.float32)
    Ub = np.asarray(Ub, np.float32)
    bb = np.asarray(bb, np.float32)

    nc = _get_program()
    in_maps = make_in_maps(xf, xb, Wf, Uf, bf, Wb, Ub, bb)
    res = run_bass_kernel_spmd(nc, in_maps, list(range(NCORE)))

    out = np.empty((B, T, 2 * U), np.float32)
    for core in range(NCORE):
        d, j = divmod(core, NDIR_CORES)
        hsv = np.asarray(res.results[core]["hsT"],
                         dtype=np.float32)  # [T, U, b_loc]
        out[B_LOC * j:B_LOC * (j + 1), :, U * d:U * (d + 1)] = \
            hsv.transpose(2, 0, 1)
    return out


# revision 4
# speedup vs baseline: 1.1116x; 1.0621x over previous
"""BiLSTM Trainium2 kernel, v4 — transposed-z, single-bank merged tail.

Like v3 (z computed transposed, U-stationary, bf16 U/h), but the whole
step's z^T lives in ONE PSUM tile [128, 256] (1 bank):
  columns = [gate block g|i|f|o] x [chunk c 0..3] x [batch 16]
  (gate-major: block gg at cols 64*gg, chunk c at 16*c within the block)
so the step's tail is ONE sigmoid [128,256], ONE tg/fc/ig/cn/hn each at
[128,64], ONE tanh, and h^T comes out as ONE [128,64] bf16 tile whose
16-col slices are exactly the next step's four matmul moving operands.
The xz inject is ONE [128,256] f32r matmul (eye stationary) that opens
the accumulation bank.
"""

import os
import sys

sys.path.insert(0, "/opt/trn_rl_repo")

import numpy as np
import ml_dtypes
from contextlib import ExitStack

import concourse.bass as bass  # noqa: F401
import concourse.tile as tile
from concourse import bacc, mybir
from concourse.bass_utils import run_bass_kernel_spmd

B, T, D, U = 64, 512, 512, 512
G = 4 * U
NCORE = 8
NDIR_CORES = 4
B_LOC = B // NDIR_CORES        # 16
NCHUNK = 4                     # h k-tiles
CH = U // NCHUNK               # 128
NS = G // 128                  # 16 gate slices

ZBUFS = int(os.environ.get("BK4_ZBUFS", "2"))
PJBUFS = int(os.environ.get("BK4_PJBUFS", "2"))
XBUFS = int(os.environ.get("BK4_XBUFS", "4"))
FC_ENG = os.environ.get("BK4_FC_ENG", "dve")
OC_ENG = os.environ.get("BK4_OC_ENG", "act2")
PRE_RT = int(os.environ.get("BK4_PRE_RT", "2"))

F32 = mybir.dt.float32
F32R = mybir.dt.float32r
BF16 = mybir.dt.bfloat16
AF = mybir.ActivationFunctionType

# gate-major slice order: slice s = 4*gg + c, gg in (g, i, f, o)
GBASE = {0: 2 * U, 1: 0, 2: U, 3: 3 * U}  # g, i, f, o original col bases


def _perm_t():
    """permT[128*s + m] = original gate column of slice s, lane m.

    Chunk-major: slice s = 4*c + gg, gg in (g, i, f, o).
    """
    idx = np.empty(G, np.int64)
    for c in range(NCHUNK):
        for gg in range(4):
            s = 4 * c + gg
            idx[128 * s:128 * (s + 1)] = GBASE[gg] + CH * c + np.arange(128)
    return idx


def _emit(tc, nc, xT, Wp, Up, biasb, eye128, zerosb, hsT, t_steps, b_loc):
    rt = t_steps * b_loc
    n_rt = rt // 512
    W4 = 4 * b_loc              # 64

    with ExitStack() as es:
        consts = es.enter_context(tc.tile_pool(name="consts", bufs=1))
        dramp = es.enter_context(tc.tile_pool(name="dram", bufs=1,
                                              space="DRAM"))

        xzT = dramp.tile([G, rt], F32R, tag="xzT")

        w_t = consts.tile([128, 4, G], F32R, tag="w")
        u_t = consts.tile([128, 4, G], BF16, tag="u")
        for k in range(4):
            nc.sync.dma_start(out=w_t[:, k, :],
                              in_=Wp[128 * k:128 * (k + 1), :])
            nc.sync.dma_start(out=u_t[:, k, :],
                              in_=Up[128 * k:128 * (k + 1), :])
        bias_t = consts.tile([128, NS], F32, tag="bias")
        nc.sync.dma_start(out=bias_t, in_=biasb)
        eye_t = consts.tile([128, 128], F32R, tag="eye")
        nc.sync.dma_start(out=eye_t, in_=eye128)

        # ---- phase 1: xz^T = (x @ W)^T, W stationary -------------------
        es2 = es.enter_context(ExitStack())
        xkp = es2.enter_context(tc.tile_pool(name="xk", bufs=3))
        pcp = es2.enter_context(tc.tile_pool(name="pc", bufs=3))
        pjps = es2.enter_context(tc.tile_pool(name="pjps", bufs=PJBUFS,
                                              space="PSUM"))
        xk_tiles = {}

        def emit_proj_s(rti, s):
            if s == 0:
                xk = xkp.tile([128, 4, 512], F32R, tag="xk",
                              name=f"xk_{rti}")
                for k in range(4):
                    nc.sync.dma_start(
                        out=xk[:, k, :],
                        in_=xT[128 * k:128 * (k + 1),
                               512 * rti:512 * (rti + 1)])
                xk_tiles[rti] = xk
            xk = xk_tiles[rti]
            pj = pjps.tile([128, 512], F32, tag="pj", name=f"pj_{rti}_{s}")
            for k in range(4):
                nc.tensor.matmul(pj, w_t[:, k, 128 * s:128 * (s + 1)],
                                 xk[:, k, :], start=(k == 0), stop=(k == 3))
            oc = pcp.tile([128, 512], F32R, tag="oc", name=f"oc_{rti}_{s}")
            use_act = OC_ENG.startswith("act") or (OC_ENG == "alt"
                                                   and s % 2 == 0)
            if use_act:
                # bias is per-partition (gate-col) in this layout, so the
                # scalar engine's activation bias applies it during the copy
                nparts = int(OC_ENG[3:]) if len(OC_ENG) > 3 else 1
                w = 512 // nparts
                for j in range(nparts):
                    nc.scalar.activation(oc[:, w * j:w * (j + 1)],
                                         pj[:, w * j:w * (j + 1)],
                                         AF.Identity,
                                         bias=bias_t[:, s:s + 1])
            else:
                nc.vector.tensor_scalar(oc, pj, bias_t[:, s:s + 1], 0.0,
                                        mybir.AluOpType.add,
                                        mybir.AluOpType.bypass)
            nc.sync.dma_start(
                out=xzT[128 * s:128 * (s + 1), 512 * rti:512 * (rti + 1)],
                in_=oc)

        PRE = min(n_rt, PRE_RT)
        for rti in range(PRE):
            for s in range(NS):
                emit_proj_s(rti, s)
        next_q = PRE * NS

        # ---- phase 2: recurrence ---------------------------------------
        with tc.tile_pool(name="xzp", bufs=XBUFS) as xzp, \
             tc.tile_pool(name="state", bufs=2) as state, \
             tc.tile_pool(name="gates", bufs=3) as gp, \
             tc.tile_pool(name="zps", bufs=ZBUFS, space="PSUM") as zps:

            W2 = 2 * b_loc      # 32: per-half h/c width
            h_prev = [state.tile([128, W2], BF16, tag=f"h{x}",
                                 name=f"h_init{x}") for x in "ab"]
            c_prev = [state.tile([128, W2], F32, tag=f"c{x}",
                                 name=f"c_init{x}") for x in "ab"]
            for x in range(2):
                nc.sync.dma_start(out=h_prev[x], in_=zerosb[:, :W2])
                nc.vector.memset(c_prev[x][:], 0.0)

            xz_tiles = {}

            def issue_xz_dma(t2):
                # one DMA loads xz for steps 2*t2 and 2*t2+1
                if 2 * t2 >= t_steps:
                    return
                nb = min(2 * b_loc, (t_steps - 2 * t2) * b_loc)
                xz_t = xzp.tile([128, NS, 2 * b_loc], F32R, tag="xz_t",
                                name=f"xzt_{t2}")
                nc.sync.dma_start(
                    out=xz_t[:, :, 0:nb],
                    in_=xzT[:, 2 * b_loc * t2:2 * b_loc * t2 + nb].rearrange(
                        "(s p) c -> p s c", p=128))
                xz_tiles[t2] = xz_t

            issue_xz_dma(0)
            issue_xz_dma(1)

            for t in range(t_steps):
                xz_t = xz_tiles[t // 2]
                if t % 2 == 1:
                    del xz_tiles[t // 2]
                half = slice(b_loc * (t % 2), b_loc * (t % 2 + 1))
                # two z banks per step: bank X holds slices of chunk pair X
                # (A: chunks 0-1 / slices 0-7; B: chunks 2-3 / slices 8-15)
                zts = [zps.tile([128, 8 * b_loc], F32, tag=f"z{x}",
                                name=f"z_{t}_{x}") for x in "ab"]
                for X in range(2):
                    nc.tensor.matmul(
                        zts[X], eye_t,
                        xz_t[:, 8 * X:8 * (X + 1), half],
                        start=True, stop=False,
                        skip_group_check=True)
                for r in range(NCHUNK):
                    last = r == NCHUNK - 1
                    hslice = h_prev[r // 2][:, b_loc * (r % 2):
                                            b_loc * (r % 2 + 1)]
                    for s in range(NS):
                        X, sl = divmod(s, 8)
                        nc.tensor.matmul(
                            zts[X][:, b_loc * sl:b_loc * (sl + 1)],
                            u_t[:, r, 128 * s:128 * (s + 1)],
                            hslice,
                            start=False, stop=(last and sl == 7),
                            skip_group_check=True)

                # tails per half: slices within a half are [g|i|f|o] per
                # chunk, so gate planes are stride-4 slice combs.
                h_new = [None, None]
                c_new = [None, None]
                for X in range(2):
                    sig = gp.tile([128, 8 * b_loc], F32, tag=f"sig{X}",
                                  name=f"sig_{t}_{X}")
                    nc.scalar.activation(sig, zts[X], AF.Sigmoid)
                    sg = sig.rearrange("p (s b) -> p s b", b=b_loc)
                    tg = gp.tile([128, W2], F32, tag=f"tg{X}",
                                 name=f"tg_{t}_{X}")
                    nc.vector.tensor_scalar(
                        tg, sg[:, 0::4, :], 2.0, 1.0,
                        mybir.AluOpType.mult, mybir.AluOpType.subtract)
                    fc = gp.tile([128, W2], F32, tag=f"fc{X}",
                                 name=f"fc_{t}_{X}")
                    fc_eng = nc.gpsimd if FC_ENG == "pool" else nc.vector
                    fc_eng.tensor_mul(fc, sg[:, 2::4, :], c_prev[X])
                    ig = gp.tile([128, W2], F32, tag=f"ig{X}",
                                 name=f"ig_{t}_{X}")
                    nc.vector.tensor_mul(ig, sg[:, 1::4, :], tg)
                    cn = state.tile([128, W2], F32, tag=f"c{'ab'[X]}",
                                    name=f"c_{t}_{X}")
                    nc.vector.tensor_add(cn, ig, fc)
                    tch = gp.tile([128, W2], F32, tag=f"tc{X}",
                                  name=f"tc_{t}_{X}")
                    nc.scalar.activation(tch, cn, AF.Tanh)
                    hn = state.tile([128, W2], BF16, tag=f"h{'ab'[X]}",
                                    name=f"h_{t}_{X}")
                    nc.vector.tensor_mul(hn, sg[:, 3::4, :], tch)
                    nc.sync.dma_start(
                        out=hsT[t, 2 * CH * X:2 * CH * (X + 1),
                                :].rearrange("(c p) b -> p c b", p=128),
                        in_=hn)
                    h_new[X] = hn
                    c_new[X] = cn

                if t % 2 == 0 and next_q < n_rt * NS:
                    emit_proj_s(next_q // NS, next_q % NS)
                    next_q += 1
                if t % 2 == 0:
                    issue_xz_dma(t // 2 + 2)
                h_prev, c_prev = h_new, c_new


def build_program(t_steps=T, b_loc=B_LOC):
    rt = t_steps * b_loc
    nc = bacc.Bacc("TRN2", target_bir_lowering=False, debug=False,
                   num_devices=NCORE)
    xT = nc.dram_tensor("xT", [D, rt], F32R, kind="ExternalInput").ap()
    Wp = nc.dram_tensor("Wp", [D, G], F32R, kind="ExternalInput").ap()
    Up = nc.dram_tensor("Up", [U, G], BF16, kind="ExternalInput").ap()
    biasb = nc.dram_tensor("biasb", [128, NS], F32,
                           kind="ExternalInput").ap()
    eye128 = nc.dram_tensor("eye128", [128, 128], F32R,
                            kind="ExternalInput").ap()
    zerosb = nc.dram_tensor("zerosb", [128, 4 * b_loc], BF16,
                            kind="ExternalInput").ap()
    hsT = nc.dram_tensor("hsT", [t_steps, U, b_loc], BF16,
                         kind="ExternalOutput").ap()
    with tile.TileContext(nc) as tc:
        _emit(tc, nc, xT, Wp, Up, biasb, eye128, zerosb, hsT,
              t_steps, b_loc)
    nc.compile()
    return nc


_CACHE = {}


def _get_program(t_steps=T, b_loc=B_LOC):
    key = (t_steps, b_loc)
    if key not in _CACHE:
        _CACHE[key] = build_program(t_steps, b_loc)
    return _CACHE[key]


def make_in_maps(xf, xb, Wf, Uf, bf, Wb, Ub, bb, t_steps=T, b_loc=B_LOC):
    perm = _perm_t()
    gscale = np.ones(G, np.float32)
    for c in range(NCHUNK):
        s = 4 * c               # g slice of chunk c
        gscale[128 * s:128 * (s + 1)] = 2.0
    packs = {}
    for d, (W, Urec, bias) in enumerate(((Wf, Uf, bf), (Wb, Ub, bb))):
        Wpp = np.ascontiguousarray(W[:, perm] * gscale)
        Upp = np.ascontiguousarray(
            (Urec[:, perm] * gscale).astype(ml_dtypes.bfloat16))
        bp = np.ascontiguousarray(
            ((bias[perm] * gscale).reshape(NS, 128).T).astype(np.float32))
        packs[d] = (Wpp, Upp, bp)
    in_maps = []
    for core in range(NCORE):
        d, j = divmod(core, NDIR_CORES)
        x = (xf if d == 0 else xb)[b_loc * j:b_loc * (j + 1), :t_steps]
        xT = np.ascontiguousarray(
            x.transpose(2, 1, 0).reshape(D, t_steps * b_loc))
        Wpp, Upp, bp = packs[d]
        in_maps.append({
            "xT": xT, "Wp": Wpp, "Up": Upp, "biasb": bp,
            "eye128": np.eye(128, dtype=np.float32),
            "zerosb": np.zeros((128, 4 * b_loc), ml_dtypes.bfloat16)})
    return in_maps


def kernel(xf, xb, Wf, Uf, bf, Wb, Ub, bb):
    xf = np.asarray(xf, np.float32)
    xb = np.asarray(xb, np.float32)
    Wf = np.asarray(Wf, np.float32)
    Uf = np.asarray(Uf, np.float32)
    bf = np.asarray(bf, np.float32)
    Wb = np.asarray(Wb, np.float32)
    Ub = np.asarray(Ub, np.float32)
    bb = np.asarray(bb, np.float32)

    nc = _get_program()
    in_maps = make_in_maps(xf, xb, Wf, Uf, bf, Wb, Ub, bb)
    res = run_bass_kernel_spmd(nc, in_maps, list(range(NCORE)))

    out = np.empty((B, T, 2 * U), np.float32)
    for core in range(NCORE):
        d, j = divmod(core, NDIR_CORES)
        hsv = np.asarray(res.results[core]["hsT"],
                         dtype=np.float32)  # [T, U, b_loc]
        out[B_LOC * j:B_LOC * (j + 1), :, U * d:U * (d + 1)] = \
            hsv.transpose(2, 0, 1)
    return out


# revision 5
# speedup vs baseline: 1.1305x; 1.0170x over previous
"""BiLSTM Trainium2 kernel, v4 — transposed-z, single-bank merged tail.

Like v3 (z computed transposed, U-stationary, bf16 U/h), but the whole
step's z^T lives in ONE PSUM tile [128, 256] (1 bank):
  columns = [gate block g|i|f|o] x [chunk c 0..3] x [batch 16]
  (gate-major: block gg at cols 64*gg, chunk c at 16*c within the block)
so the step's tail is ONE sigmoid [128,256], ONE tg/fc/ig/cn/hn each at
[128,64], ONE tanh, and h^T comes out as ONE [128,64] bf16 tile whose
16-col slices are exactly the next step's four matmul moving operands.
The xz inject is ONE [128,256] f32r matmul (eye stationary) that opens
the accumulation bank.
"""

import os
import sys

sys.path.insert(0, "/opt/trn_rl_repo")

import numpy as np
import ml_dtypes
from contextlib import ExitStack

import concourse.bass as bass  # noqa: F401
import concourse.tile as tile
from concourse import bacc, mybir
from concourse.bass_utils import run_bass_kernel_spmd

B, T, D, U = 64, 512, 512, 512
G = 4 * U
NCORE = 8
NDIR_CORES = 4
B_LOC = B // NDIR_CORES        # 16
NCHUNK = 4                     # h k-tiles
CH = U // NCHUNK               # 128
NS = G // 128                  # 16 gate slices

ZBUFS = int(os.environ.get("BK4_ZBUFS", "2"))
PJBUFS = int(os.environ.get("BK4_PJBUFS", "2"))
XBUFS = int(os.environ.get("BK4_XBUFS", "4"))
FC_ENG = os.environ.get("BK4_FC_ENG", "dve")
OC_ENG = os.environ.get("BK4_OC_ENG", "act2")
PRE_RT = int(os.environ.get("BK4_PRE_RT", "2"))
PROJ_PH = int(os.environ.get("BK4_PROJ_PH", "1"))

F32 = mybir.dt.float32
F32R = mybir.dt.float32r
BF16 = mybir.dt.bfloat16
AF = mybir.ActivationFunctionType

# gate-major slice order: slice s = 4*gg + c, gg in (g, i, f, o)
GBASE = {0: 2 * U, 1: 0, 2: U, 3: 3 * U}  # g, i, f, o original col bases


def _perm_t():
    """permT[128*s + m] = original gate column of slice s, lane m.

    Chunk-major: slice s = 4*c + gg, gg in (g, i, f, o).
    """
    idx = np.empty(G, np.int64)
    for c in range(NCHUNK):
        for gg in range(4):
            s = 4 * c + gg
            idx[128 * s:128 * (s + 1)] = GBASE[gg] + CH * c + np.arange(128)
    return idx


def _emit(tc, nc, xT, Wp, Up, biasb, eye128, zerosb, hsT, t_steps, b_loc):
    rt = t_steps * b_loc
    n_rt = rt // 512
    W4 = 4 * b_loc              # 64

    with ExitStack() as es:
        consts = es.enter_context(tc.tile_pool(name="consts", bufs=1))
        dramp = es.enter_context(tc.tile_pool(name="dram", bufs=1,
                                              space="DRAM"))

        xzT = dramp.tile([G, rt], F32R, tag="xzT")

        w_t = consts.tile([128, 4, G], F32R, tag="w")
        u_t = consts.tile([128, 4, G], BF16, tag="u")
        for k in range(4):
            nc.sync.dma_start(out=w_t[:, k, :],
                              in_=Wp[128 * k:128 * (k + 1), :])
            nc.sync.dma_start(out=u_t[:, k, :],
                              in_=Up[128 * k:128 * (k + 1), :])
        bias_t = consts.tile([128, NS], F32, tag="bias")
        nc.sync.dma_start(out=bias_t, in_=biasb)
        eye_t = consts.tile([128, 128], F32R, tag="eye")
        nc.sync.dma_start(out=eye_t, in_=eye128)

        # ---- phase 1: xz^T = (x @ W)^T, W stationary -------------------
        es2 = es.enter_context(ExitStack())
        xkp = es2.enter_context(tc.tile_pool(name="xk", bufs=3))
        pcp = es2.enter_context(tc.tile_pool(name="pc", bufs=3))
        pjps = es2.enter_context(tc.tile_pool(name="pjps", bufs=PJBUFS,
                                              space="PSUM"))
        xk_tiles = {}

        def emit_proj_s(rti, s):
            if s == 0:
                xk = xkp.tile([128, 4, 512], F32R, tag="xk",
                              name=f"xk_{rti}")
                for k in range(4):
                    nc.sync.dma_start(
                        out=xk[:, k, :],
                        in_=xT[128 * k:128 * (k + 1),
                               512 * rti:512 * (rti + 1)])
                xk_tiles[rti] = xk
            xk = xk_tiles[rti]
            pj = pjps.tile([128, 512], F32, tag="pj", name=f"pj_{rti}_{s}")
            for k in range(4):
                nc.tensor.matmul(pj, w_t[:, k, 128 * s:128 * (s + 1)],
                                 xk[:, k, :], start=(k == 0), stop=(k == 3))
            oc = pcp.tile([128, 512], F32R, tag="oc", name=f"oc_{rti}_{s}")
            use_act = OC_ENG.startswith("act") or (OC_ENG == "alt"
                                                   and s % 2 == 0)
            if use_act:
                # bias is per-partition (gate-col) in this layout, so the
                # scalar engine's activation bias applies it during the copy
                nparts = int(OC_ENG[3:]) if len(OC_ENG) > 3 else 1
                w = 512 // nparts
                for j in range(nparts):
                    nc.scalar.activation(oc[:, w * j:w * (j + 1)],
                                         pj[:, w * j:w * (j + 1)],
                                         AF.Identity,
                                         bias=bias_t[:, s:s + 1])
            else:
                nc.vector.tensor_scalar(oc, pj, bias_t[:, s:s + 1], 0.0,
                                        mybir.AluOpType.add,
                                        mybir.AluOpType.bypass)
            nc.sync.dma_start(
                out=xzT[128 * s:128 * (s + 1), 512 * rti:512 * (rti + 1)],
                in_=oc)

        PRE = min(n_rt, PRE_RT)
        for rti in range(PRE):
            for s in range(NS):
                emit_proj_s(rti, s)
        next_q = PRE * NS

        # ---- phase 2: recurrence ---------------------------------------
        with tc.tile_pool(name="xzp", bufs=XBUFS) as xzp, \
             tc.tile_pool(name="state", bufs=2) as state, \
             tc.tile_pool(name="gates", bufs=3) as gp, \
             tc.tile_pool(name="zps", bufs=ZBUFS, space="PSUM") as zps:

            W2 = 2 * b_loc      # 32: per-half h/c width
            h_prev = [state.tile([128, W2], BF16, tag=f"h{x}",
                                 name=f"h_init{x}") for x in "ab"]
            c_prev = [state.tile([128, W2], F32, tag=f"c{x}",
                                 name=f"c_init{x}") for x in "ab"]
            for x in range(2):
                nc.sync.dma_start(out=h_prev[x], in_=zerosb[:, :W2])
                nc.vector.memset(c_prev[x][:], 0.0)

            xz_tiles = {}

            def issue_xz_dma(t2):
                # one DMA loads xz for steps 2*t2 and 2*t2+1
                if 2 * t2 >= t_steps:
                    return
                nb = min(2 * b_loc, (t_steps - 2 * t2) * b_loc)
                xz_t = xzp.tile([128, NS, 2 * b_loc], F32R, tag="xz_t",
                                name=f"xzt_{t2}")
                nc.sync.dma_start(
                    out=xz_t[:, :, 0:nb],
                    in_=xzT[:, 2 * b_loc * t2:2 * b_loc * t2 + nb].rearrange(
                        "(s p) c -> p s c", p=128))
                xz_tiles[t2] = xz_t

            issue_xz_dma(0)
            issue_xz_dma(1)

            for t in range(t_steps):
                xz_t = xz_tiles[t // 2]
                if t % 2 == 1:
                    del xz_tiles[t // 2]
                half = slice(b_loc * (t % 2), b_loc * (t % 2 + 1))
                # two z banks per step: bank X holds slices of chunk pair X
                # (A: chunks 0-1 / slices 0-7; B: chunks 2-3 / slices 8-15)
                zts = [zps.tile([128, 8 * b_loc], F32, tag=f"z{x}",
                                name=f"z_{t}_{x}") for x in "ab"]
                for X in range(2):
                    nc.tensor.matmul(
                        zts[X], eye_t,
                        xz_t[:, 8 * X:8 * (X + 1), half],
                        start=True, stop=False,
                        skip_group_check=True)
                for r in range(NCHUNK):
                    last = r == NCHUNK - 1
                    hslice = h_prev[r // 2][:, b_loc * (r % 2):
                                            b_loc * (r % 2 + 1)]
                    for s in range(NS):
                        X, sl = divmod(s, 8)
                        nc.tensor.matmul(
                            zts[X][:, b_loc * sl:b_loc * (sl + 1)],
                            u_t[:, r, 128 * s:128 * (s + 1)],
                            hslice,
                            start=False, stop=(last and sl == 7),
                            skip_group_check=True)

                # tails per half: slices within a half are [g|i|f|o] per
                # chunk, so gate planes are stride-4 slice combs.
                h_new = [None, None]
                c_new = [None, None]
                for X in range(2):
                    sig = gp.tile([128, 8 * b_loc], F32, tag=f"sig{X}",
                                  name=f"sig_{t}_{X}")
                    nc.scalar.activation(sig, zts[X], AF.Sigmoid)
                    sg = sig.rearrange("p (s b) -> p s b", b=b_loc)
                    tg = gp.tile([128, W2], F32, tag=f"tg{X}",
                                 name=f"tg_{t}_{X}")
                    nc.vector.tensor_scalar(
                        tg, sg[:, 0::4, :], 2.0, 1.0,
                        mybir.AluOpType.mult, mybir.AluOpType.subtract)
                    fc = gp.tile([128, W2], F32, tag=f"fc{X}",
                                 name=f"fc_{t}_{X}")
                    fc_eng = nc.gpsimd if FC_ENG == "pool" else nc.vector
                    fc_eng.tensor_mul(fc, sg[:, 2::4, :], c_prev[X])
                    ig = gp.tile([128, W2], F32, tag=f"ig{X}",
                                 name=f"ig_{t}_{X}")
                    nc.vector.tensor_mul(ig, sg[:, 1::4, :], tg)
                    cn = state.tile([128, W2], F32, tag=f"c{'ab'[X]}",
                                    name=f"c_{t}_{X}")
                    nc.vector.tensor_add(cn, ig, fc)
                    tch = gp.tile([128, W2], F32, tag=f"tc{X}",
                                  name=f"tc_{t}_{X}")
                    nc.scalar.activation(tch, cn, AF.Tanh)
                    hn = state.tile([128, W2], BF16, tag=f"h{'ab'[X]}",
                                    name=f"h_{t}_{X}")
                    nc.vector.tensor_mul(hn, sg[:, 3::4, :], tch)
                    nc.sync.dma_start(
                        out=hsT[t, 2 * CH * X:2 * CH * (X + 1),
                                :].rearrange("(c p) b -> p c b", p=128),
                        in_=hn)
                    h_new[X] = hn
                    c_new[X] = cn

                if t % 2 == PROJ_PH and next_q < n_rt * NS:
                    emit_proj_s(next_q // NS, next_q % NS)
                    next_q += 1
                if t % 2 == 0:
                    issue_xz_dma(t // 2 + 2)
                h_prev, c_prev = h_new, c_new


def build_program(t_steps=T, b_loc=B_LOC):
    rt = t_steps * b_loc
    nc = bacc.Bacc("TRN2", target_bir_lowering=False, debug=False,
                   num_devices=NCORE)
    xT = nc.dram_tensor("xT", [D, rt], F32R, kind="ExternalInput").ap()
    Wp = nc.dram_tensor("Wp", [D, G], F32R, kind="ExternalInput").ap()
    Up = nc.dram_tensor("Up", [U, G], BF16, kind="ExternalInput").ap()
    biasb = nc.dram_tensor("biasb", [128, NS], F32,
                           kind="ExternalInput").ap()
    eye128 = nc.dram_tensor("eye128", [128, 128], F32R,
                            kind="ExternalInput").ap()
    zerosb = nc.dram_tensor("zerosb", [128, 4 * b_loc], BF16,
                            kind="ExternalInput").ap()
    hsT = nc.dram_tensor("hsT", [t_steps, U, b_loc], BF16,
                         kind="ExternalOutput").ap()
    with tile.TileContext(nc) as tc:
        _emit(tc, nc, xT, Wp, Up, biasb, eye128, zerosb, hsT,
              t_steps, b_loc)
    nc.compile()
    return nc


_CACHE = {}


def _get_program(t_steps=T, b_loc=B_LOC):
    key = (t_steps, b_loc)
    if key not in _CACHE:
        _CACHE[key] = build_program(t_steps, b_loc)
    return _CACHE[key]


def make_in_maps(xf, xb, Wf, Uf, bf, Wb, Ub, bb, t_steps=T, b_loc=B_LOC):
    perm = _perm_t()
    gscale = np.ones(G, np.float32)
    for c in range(NCHUNK):
        s = 4 * c               # g slice of chunk c
        gscale[128 * s:128 * (s + 1)] = 2.0
    packs = {}
    for d, (W, Urec, bias) in enumerate(((Wf, Uf, bf), (Wb, Ub, bb))):
        Wpp = np.ascontiguousarray(W[:, perm] * gscale)
        Upp = np.ascontiguousarray(
            (Urec[:, perm] * gscale).astype(ml_dtypes.bfloat16))
        bp = np.ascontiguousarray(
            ((bias[perm] * gscale).reshape(NS, 128).T).astype(np.float32))
        packs[d] = (Wpp, Upp, bp)
    in_maps = []
    for core in range(NCORE):
        d, j = divmod(core, NDIR_CORES)
        x = (xf if d == 0 else xb)[b_loc * j:b_loc * (j + 1), :t_steps]
        xT = np.ascontiguousarray(
            x.transpose(2, 1, 0).reshape(D, t_steps * b_loc))
        Wpp, Upp, bp = packs[d]
        in_maps.append({
            "xT": xT, "Wp": Wpp, "Up": Upp, "biasb": bp,
            "eye128": np.eye(128, dtype=np.float32),
            "zerosb": np.zeros((128, 4 * b_loc), ml_dtypes.bfloat16)})
    return in_maps


def kernel(xf, xb, Wf, Uf, bf, Wb, Ub, bb):
    xf = np.asarray(xf, np.float32)
    xb = np.asarray(xb, np.float32)
    Wf = np.asarray(Wf, np.float32)
    Uf = np.asarray(Uf, np.float32)
    bf = np.asarray(bf, np.float32)
    Wb = np.asarray(Wb, np.float32)
    Ub = np.asarray(Ub, np.float32)
    bb = np.asarray(bb, np.float32)

    nc = _get_program()
    in_maps = make_in_maps(xf, xb, Wf, Uf, bf, Wb, Ub, bb)
    res = run_bass_kernel_spmd(nc, in_maps, list(range(NCORE)))

    out = np.empty((B, T, 2 * U), np.float32)
    for core in range(NCORE):
        d, j = divmod(core, NDIR_CORES)
        hsv = np.asarray(res.results[core]["hsT"],
                         dtype=np.float32)  # [T, U, b_loc]
        out[B_LOC * j:B_LOC * (j + 1), :, U * d:U * (d + 1)] = \
            hsv.transpose(2, 0, 1)
    return out
